# revision 16
# baseline (speedup 1.0000x reference)
"""Bass/Trainium2 kernel for BaseWindowAttention (8x8 windows, 8 heads, dim 256).

Data-parallel over 8 NeuronCores: each core processes one (b, l) image of
[128, 128, 256]. Fully fused on-device pipeline: qkv projection -> windowed
attention (64-token windows) -> output projection.

Layout strategy per core:
  - tokens are processed in "groups" of 512 = 4 window-pairs (wp = 2 adjacent
    8x8 windows = 128 tokens, partition order (w, r, c)).
  - x is pre-transposed on host to [32 groups, 128 ch, (2 ch-half, 512 tok)]
    so the contraction dim (channels) lands on SBUF partitions with zero
    on-chip transposes for the projection stage.
  - qT/kT live as [outch(4 heads x 32d) partitions, tok free] -> attention
    score matmuls S^T = kT^T @ qT slice straight out of SBUF via
    tile_position packing (K=32, M=64, N=64).
  - the 64x64 relative-position bias is ADDED into the scores PSUM by one
    extra matmul per (j, h4-bank): stationary = tiled bias^T, moving = tiled
    identity. The exp() then directly produces E = exp(S + B), removing the
    separate bias multiply from the critical path.
  - AV uses E as the stationary operand and V-natural as moving, with a ones
    column appended to V so each window-head matmul also emits the softmax
    denominator. Output lands token-major -> normalize is a native
    per-partition broadcast multiply.
  - O is PE-transposed back to [ch, tok]; the out projection runs
    weight-stationary (W_out^T slices) producing ch-major output written
    straight to DRAM by one DMA per group.
  - the group loop is software-pipelined: group g+1's qkv projections are
    emitted interleaved with group g's attention so the in-order engine
    streams (PE/ACT/DVE) have fill work while the per-j dependency chain
    (scores -> exp -> AV -> normalize -> transpose) crosses engines.
"""

import os
import numpy as np
import ml_dtypes

import concourse.bass as bass
import concourse.bacc as bacc
import concourse.mybir as mybir
import concourse.tile as tile
from concourse.bass_utils import run_bass_kernel_spmd
from contextlib import ExitStack

F32 = mybir.dt.float32
BF16 = mybir.dt.bfloat16

WS = 8
HEADS = 8
HD = 32
DIM = 256
STRIDE = 2 * WS - 1
SCALE = HD ** -0.5
N_CORES = 8
NG_FULL = 32  # 512-token groups per core

BF = ml_dtypes.bfloat16


def _bias_table() -> np.ndarray:
    # bias[qi, kj] from the 15x15 pos table, same as the reference
    coords = np.array([[x, y] for x in range(WS) for y in range(WS)], dtype=np.int32)
    rel = coords[None, :, :] - coords[:, None, :] + (WS - 1)
    idx = rel[:, :, 0] * STRIDE + rel[:, :, 1]
    return np.clip(idx, 0, None).reshape(WS * WS, WS * WS)


def build(n_groups: int = NG_FULL):
    nc = bacc.Bacc("TRN2", target_bir_lowering=False, debug=False,
                   num_devices=N_CORES)

    xT = nc.dram_tensor("xT", [NG_FULL, 128, 1024], BF16, kind="ExternalInput")
    wt = nc.dram_tensor("wt", [128, 1536], BF16, kind="ExternalInput")
    wo = nc.dram_tensor("wo", [128, 512], BF16, kind="ExternalInput")
    bt2 = nc.dram_tensor("bt2", [64, 128], BF16, kind="ExternalInput")
    i2 = nc.dram_tensor("i2", [64, 128], BF16, kind="ExternalInput")
    idn = nc.dram_tensor("idn", [128, 128], BF16, kind="ExternalInput")
    # output in kernel visit order [g, outch-half(p), (oh, tok)]; host
    # inverse-permutes to the natural image layout
    out = nc.dram_tensor("out", [NG_FULL, 128, 1024], BF16,
                         kind="ExternalOutput")

    with tile.TileContext(nc) as tc, ExitStack() as ctx:
        consts = ctx.enter_context(tc.tile_pool(name="consts", bufs=1))
        wt_s = consts.tile([128, 1536], BF16)
        wo_s = consts.tile([128, 512], BF16)
        bt2_s = consts.tile([64, 128], BF16)
        i2_s = consts.tile([64, 128], BF16)
        idn_s = consts.tile([128, 128], BF16)
        nc.sync.dma_start(wt_s, wt.ap())
        nc.sync.dma_start(wo_s, wo.ap())
        nc.sync.dma_start(bt2_s, bt2.ap())
        nc.sync.dma_start(i2_s, i2.ap())
        nc.sync.dma_start(idn_s, idn.ap())

        # sbuf pools
        xt_p = ctx.enter_context(tc.tile_pool(name="xt", bufs=3))
        qk_p = ctx.enter_context(tc.tile_pool(name="qksb", bufs=4))
        v_p = ctx.enter_context(tc.tile_pool(name="vsb", bufs=3))
        e_p = ctx.enter_context(tc.tile_pool(name="esb", bufs=6))
        onm_p = ctx.enter_context(tc.tile_pool(name="onm", bufs=6))
        rc_p = ctx.enter_context(tc.tile_pool(name="rc", bufs=6))
        ot_p = ctx.enter_context(tc.tile_pool(name="ot", bufs=3))
        ob_p = ctx.enter_context(tc.tile_pool(name="ob", bufs=4))

        # psum pools (8 banks of [128, 2KB] total)
        # qkv: [128,512] slots, 2 banks; sp: 4 banks (one per PE row-group --
        # concurrent row-tiled matmuls must write different banks or the
        # device dies); tail: shared 1-bank slots for AV out / O-transpose /
        # out-proj
        qkv_ps = ctx.enter_context(tc.tile_pool(name="qkvps", bufs=2, space="PSUM"))
        sp_ps = ctx.enter_context(tc.tile_pool(name="spps", bufs=1, space="PSUM"))
        tail_ps = ctx.enter_context(tc.tile_pool(name="tailps", bufs=2, space="PSUM"))

        st = {}  # per-group live tiles

        def emit_dma_in(g):
            xt01 = xt_p.tile([128, 1024], BF16, tag="xt01")
            nc.sync.dma_start(xt01, xT.ap()[g])
            st[g] = {"xt": xt01}

        def emit_proj_qk(g, which):  # which: 0 -> q (mt 0,1), 1 -> k (mt 2,3)
            xt01 = st[g]["xt"]
            xtr = [xt01[:, 0:512], xt01[:, 512:1024]]
            dst = qk_p.tile([128, 1024], BF16, tag="qt" if which == 0 else "kt")
            for mt in (2 * which, 2 * which + 1):
                mps = qkv_ps.tile([128, 512], F32, tag="qkvp")
                for kh in range(2):
                    nc.tensor.matmul(
                        mps,
                        wt_s[:, 768 * kh + 128 * mt:768 * kh + 128 * mt + 128],
                        xtr[kh], start=(kh == 0), stop=(kh == 1))
                nc.scalar.activation(
                    dst[:, 512 * (mt % 2):512 * (mt % 2) + 512], mps,
                    mybir.ActivationFunctionType.Copy)
            st[g]["qt" if which == 0 else "kt"] = dst

        def emit_proj_v(g):
            xt01 = st[g]["xt"]
            xtr = [xt01[:, 0:512], xt01[:, 512:1024]]
            va = v_p.tile([128, 1056], BF16, tag="va")  # (t4, h8, 33)
            va_r = va.rearrange("p (t h c) -> p t h c", t=4, h=8, c=33)
            for vh in range(2):  # two tok-tile pairs
                vps = qkv_ps.tile([128, 512], F32, tag="qkvp")
                for t2 in range(2):
                    t = 2 * vh + t2
                    for kh in range(2):
                        nc.tensor.matmul(
                            vps[:, 256 * t2:256 * t2 + 256],
                            xtr[kh][:, 128 * t:128 * t + 128],
                            wt_s[:, 768 * kh + 512:768 * kh + 768],
                            start=(kh == 0), stop=(kh == 1))
                vps_r = vps.rearrange("p (t h c) -> p t h c", t=2, h=8, c=32)
                nc.vector.tensor_copy(va_r[:, 2 * vh:2 * vh + 2, :, 0:32], vps_r)
            nc.gpsimd.memset(va_r[:, :, :, 32], 1.0)
            st[g]["va"] = va

        def emit_scores(g, j):
            qt, kt = st[g]["qt"], st[g]["kt"]
            # scores S^T[(w,kj), (h4-bank: hh,qi)] for window pair j.
            # one psum BANK per PE row-group (h4): concurrent row-tiled
            # matmuls into one bank are fatal on TRN2.
            sp = sp_ps.tile([128, 2048], F32, tag="sp")
            # bias add: stationary = tiled bias^T, moving = tiled identity;
            # writes B[kj, qi] into each h4-bank's used region, then the
            # score matmuls accumulate on top.
            for h4 in range(4):
                nc.tensor.matmul(sp[:, 512 * h4:512 * h4 + 128],
                                 bt2_s, i2_s, start=True, stop=False,
                                 skip_group_check=True)
            for h in range(HEADS):
                hh, h4 = divmod(h, 4)
                for w in range(2):
                    col = 512 * hh + 128 * j + 64 * w
                    nc.tensor.matmul(
                        sp[64 * w:64 * w + 64,
                           512 * h4 + 64 * hh:512 * h4 + 64 * hh + 64],
                        kt[32 * h4:32 * h4 + 32, col:col + 64],
                        qt[32 * h4:32 * h4 + 32, col:col + 64],
                        start=False, stop=(hh == 1),
                        tile_position=(32 * h4, 64 * w),
                        # the sim's zero-region tracker drops the partition
                        # base (w=1 rows check against rows 0:64) -- the
                        # bias matmul marks every region first, so the
                        # accumulate pattern is sound on hardware
                        skip_group_check=True)

            # E = exp(S + B), split in two so AV can start on the first half
            et = e_p.tile([128, 512], BF16, tag="et")
            sp_r = sp.rearrange("p (h4 r) -> p h4 r", h4=4)
            et_r = et.rearrange("p (hp h4 r) -> p hp h4 r", hp=2, h4=2)
            for half in range(2):
                nc.scalar.activation(
                    et_r[:, half], sp_r[:, 2 * half:2 * half + 2, 0:128],
                    mybir.ActivationFunctionType.Exp)
            st[g][f"et{j}"] = et

        def emit_av(g, j):
            et = st[g][f"et{j}"]
            va = st[g]["va"]
            va_r = va.rearrange("p (t h c) -> p t h c", t=4, h=8, c=33)
            # AV with ones-augmented V: O[(w,qi), (h, 32d+denom)]
            on = tail_ps.tile([128, 512], F32, tag="tail")
            for h in range(HEADS):
                hh, h4 = divmod(h, 4)
                ecol = 64 * (2 * h4 + hh)
                for w in range(2):
                    nc.tensor.matmul(
                        on[64 * w:64 * w + 64, 33 * h:33 * h + 33],
                        et[64 * w:64 * w + 64, ecol:ecol + 64],
                        va_r[64 * w:64 * w + 64, j, h, :],
                        start=True, stop=True,
                        tile_position=(64 * w, 64 * w))

            on_r = on[:, 0:264].rearrange("p (h c) -> p h c", h=8, c=33)
            rc = rc_p.tile([128, 8], F32, tag="rc")
            nc.vector.reciprocal(rc, on_r[:, :, 32])
            onm = onm_p.tile([128, 256], BF16, tag="onm")
            onm_r = onm.rearrange("p (h c) -> p h c", h=8, c=32)
            in0, in1 = bass.broadcast_tensor_aps(
                on_r[:, :, 0:32], rc.rearrange("p (h o) -> p h o", o=1))
            nc.vector.tensor_tensor(onm_r, in0, in1, op=mybir.AluOpType.mult)
            st[g][f"onm{j}"] = onm

        def emit_tr(g, j):
            onm = st[g][f"onm{j}"]
            if "ot" not in st[g]:
                st[g]["ot"] = ot_p.tile([128, 1024], BF16, tag="ot",
                                        name="ot")
            ot = st[g]["ot"]
            otp = tail_ps.tile([128, 256], BF16, tag="tail")
            for ch_half in range(2):
                nc.tensor.transpose(
                    otp[:, 128 * ch_half:128 * ch_half + 128],
                    onm[:, 128 * ch_half:128 * ch_half + 128],
                    idn_s)
            ot_r = ot.rearrange("p (chh tok) -> p chh tok", chh=2)
            otp_r = otp.rearrange("p (chh tok) -> p chh tok", chh=2)
            nc.vector.tensor_copy(ot_r[:, :, 128 * j:128 * j + 128], otp_r)

        def emit_outproj(g):
            ot = st[g]["ot"]
            ot_r = ot.rearrange("p (chh tok) -> p chh tok", chh=2)
            ob = ob_p.tile([128, 1024], BF16, tag="ob")
            for oh in range(2):
                op = tail_ps.tile([128, 512], F32, tag="tail")
                for kh in range(2):
                    nc.tensor.matmul(
                        op,
                        wo_s[:, 256 * kh + 128 * oh:256 * kh + 128 * oh + 128],
                        ot_r[:, kh, :],
                        start=(kh == 0), stop=(kh == 1))
                if oh == 0:
                    nc.scalar.activation(ob[:, 0:512], op,
                                         mybir.ActivationFunctionType.Copy)
                else:
                    nc.vector.tensor_copy(ob[:, 512:1024], op)
            nc.sync.dma_start(out.ap()[g], ob)
            del st[g]

        # ---- software-pipelined group loop
        emit_dma_in(0)
        emit_proj_qk(0, 0)
        emit_proj_qk(0, 1)
        emit_proj_v(0)
        for g in range(n_groups):
            nxt = g + 1
            if nxt < n_groups:
                emit_dma_in(nxt)
            for j in range(4):
                emit_scores(g, j)
                if j >= 1:
                    emit_av(g, j - 1)
                if j >= 2:
                    emit_tr(g, j - 2)
                if nxt < n_groups:
                    if j == 0:
                        emit_proj_qk(nxt, 0)
                    elif j == 1:
                        emit_proj_qk(nxt, 1)
                    elif j == 2:
                        emit_proj_v(nxt)
            emit_av(g, 3)
            emit_tr(g, 2)
            emit_tr(g, 3)
            emit_outproj(g)

    nc.compile()
    return nc


def _host_prep(x, W_qkv, W_out, b_out, pos_emb):
    b, l, H, W, _ = x.shape
    # xT: per core [32, 128, (kh 2, 512)] channel-major, window-pair tok order
    xr = x.reshape(b * l, 16, WS, 2, 4, 2, WS, 2, 128)
    # dims: [core, wr, r, half, j, w, c, kh, p]
    xt = np.ascontiguousarray(xr.transpose(0, 1, 3, 8, 7, 4, 5, 2, 6))
    # -> [core, wr, half, p, kh, j, w, r, c]
    xt = xt.reshape(b * l, NG_FULL, 128, 1024).astype(BF)

    wq = np.concatenate([W_qkv[:, :256] * SCALE, W_qkv[:, 256:]], axis=1)
    wt = np.ascontiguousarray(
        wq.reshape(2, 128, 768).transpose(1, 0, 2).reshape(128, 1536))

    wo = np.ascontiguousarray(
        W_out.reshape(2, 128, 256).transpose(1, 0, 2).reshape(128, 512)
    ).astype(BF)

    bias = pos_emb.reshape(-1)[_bias_table().reshape(-1)].reshape(64, 64)
    # scores land transposed (S^T[kj, qi]); out[m, n] = bt2[n%64, m] must be
    # bias[qi, kj], so bt2[c, m] = bias[c, m%64]
    bt2 = np.tile(bias.astype(np.float32), (1, 2)).astype(BF)  # [64, 128]
    i2 = np.tile(np.eye(64, dtype=np.float32), (1, 2)).astype(BF)  # [64, 128]

    idn = np.eye(128, dtype=np.float32).astype(BF)
    return xt, wt.astype(BF), wo, bt2, i2, idn


_NC_CACHE = {}


def kernel(x, W_qkv, W_out, b_out, pos_emb):
    x = np.asarray(x, dtype=np.float32)
    W_qkv = np.asarray(W_qkv, dtype=np.float32)
    W_out = np.asarray(W_out, dtype=np.float32)
    b_out = np.asarray(b_out, dtype=np.float32)
    pos_emb = np.asarray(pos_emb, dtype=np.float32)

    b, l, H, W, _ = x.shape
    xt, wt, wo, bt2, i2, idn = _host_prep(x, W_qkv, W_out, b_out, pos_emb)

    if "nc" not in _NC_CACHE:
        _NC_CACHE["nc"] = build()
    nc = _NC_CACHE["nc"]

    in_maps = [
        {"xT": np.ascontiguousarray(xt[i]), "wt": wt, "wo": wo,
         "bt2": bt2, "i2": i2, "idn": idn}
        for i in range(N_CORES)
    ]
    res = run_bass_kernel_spmd(
        nc, in_maps, list(range(N_CORES)),
        trace=bool(int(os.environ.get("KERNEL_TRACE", "0"))))
    if res.exec_time_ns is not None:
        print(f"HW exec time: {res.exec_time_ns} ns")
    outs = np.stack([res.results[i]["out"] for i in range(N_CORES)])
    return (_unscramble(outs) + b_out).reshape(b, l, H, W, DIM)


def _unscramble(o):
    # [cores, 32(wr,half), 128(outch-in-half p), 1024(oh, j, w, r, c)]
    #   -> [cores, 128, 128, 256]
    n = o.shape[0]
    o = o.astype(np.float32).reshape(n, 16, 2, 128, 2, 4, 2, WS, WS)
    # dims: n, wr, half, p, oh, j, w, r, c
    o = o.transpose(0, 1, 7, 2, 5, 6, 8, 4, 3)  # n wr r half j w c oh p
    return np.ascontiguousarray(o).reshape(n, 128, 128, DIM)


if __name__ == "__main__":
    # quick smoke: run on hardware with random inputs
    rng = np.random.default_rng(0)
    x = rng.standard_normal((2, 4, 128, 128, 256), dtype=np.float32)
    W_qkv = rng.standard_normal((256, 768), dtype=np.float32) * DIM ** -0.5
    W_out = rng.standard_normal((256, 256), dtype=np.float32) * 256 ** -0.5
    b_out = rng.standard_normal(256, dtype=np.float32) * 0.02
    pos_emb = rng.standard_normal((15, 15), dtype=np.float32)
    o = kernel(x=x, W_qkv=W_qkv, W_out=W_out, b_out=b_out, pos_emb=pos_emb)
    print(o.shape, o.dtype)


# revision 21
# speedup vs baseline: 1.3509x; 1.3509x over previous
"""Bass/Trainium2 kernel for BaseWindowAttention (8x8 windows, 8 heads, dim 256).

Data-parallel over 8 NeuronCores: each core processes one (b, l) image of
[128, 128, 256]. Fully fused on-device pipeline: qkv projection -> windowed
attention (64-token windows) -> output projection.

Layout strategy per core:
  - tokens are processed in "groups" of 512 = 4 window-pairs (wp = 2 adjacent
    8x8 windows = 128 tokens, partition order (w, r, c)).
  - x is pre-transposed on host to [32 groups, 128 ch, (2 ch-half, 512 tok)]
    so the contraction dim (channels) lands on SBUF partitions with zero
    on-chip transposes for the projection stage.
  - qT/kT live as [outch(4 heads x 32d) partitions, tok free] -> attention
    score matmuls S^T = kT^T @ qT slice straight out of SBUF via
    tile_position packing (K=32, M=64, N=64).
  - the 64x64 relative-position bias is ADDED into the scores PSUM by one
    extra matmul per (j, h4-bank): stationary = tiled bias^T, moving = tiled
    identity. The exp() then directly produces E = exp(S + B), removing the
    separate bias multiply from the critical path.
  - AV uses E as the stationary operand and V-natural as moving, with a ones
    column appended to V so each window-head matmul also emits the softmax
    denominator. Output lands token-major -> normalize is a native
    per-partition broadcast multiply.
  - O is PE-transposed back to [ch, tok]; the out projection runs
    weight-stationary (W_out^T slices) producing ch-major output written
    straight to DRAM by one DMA per group.
  - the group loop is software-pipelined: group g+1's qkv projections are
    emitted interleaved with group g's attention so the in-order engine
    streams (PE/ACT/DVE) have fill work while the per-j dependency chain
    (scores -> exp -> AV -> normalize -> transpose) crosses engines.
"""

import os
import numpy as np
import ml_dtypes

import concourse.bass as bass
import concourse.bacc as bacc
import concourse.mybir as mybir
import concourse.tile as tile
from concourse.bass_utils import run_bass_kernel_spmd
from contextlib import ExitStack

F32 = mybir.dt.float32
BF16 = mybir.dt.bfloat16

WS = 8
HEADS = 8
HD = 32
DIM = 256
STRIDE = 2 * WS - 1
SCALE = HD ** -0.5
N_CORES = 8
NG_FULL = 32  # 512-token groups per core

BF = ml_dtypes.bfloat16


def _bias_table() -> np.ndarray:
    # bias[qi, kj] from the 15x15 pos table, same as the reference
    coords = np.array([[x, y] for x in range(WS) for y in range(WS)], dtype=np.int32)
    rel = coords[None, :, :] - coords[:, None, :] + (WS - 1)
    idx = rel[:, :, 0] * STRIDE + rel[:, :, 1]
    return np.clip(idx, 0, None).reshape(WS * WS, WS * WS)


def build(n_groups: int = NG_FULL):
    nc = bacc.Bacc("TRN2", target_bir_lowering=False, debug=False,
                   num_devices=N_CORES)

    xT = nc.dram_tensor("xT", [NG_FULL, 128, 1024], BF16, kind="ExternalInput")
    wt = nc.dram_tensor("wt", [128, 1536], BF16, kind="ExternalInput")
    wo = nc.dram_tensor("wo", [128, 512], BF16, kind="ExternalInput")
    eb = nc.dram_tensor("eb", [128, 512], BF16, kind="ExternalInput")
    idn = nc.dram_tensor("idn", [128, 128], BF16, kind="ExternalInput")
    # output in kernel visit order [g, outch-half(p), (oh, tok)]; host
    # inverse-permutes to the natural image layout
    out = nc.dram_tensor("out", [NG_FULL, 128, 1024], BF16,
                         kind="ExternalOutput")

    with tile.TileContext(nc) as tc, ExitStack() as ctx:
        consts = ctx.enter_context(tc.tile_pool(name="consts", bufs=1))
        wt_s = consts.tile([128, 1536], BF16)
        wo_s = consts.tile([128, 512], BF16)
        eb_s = consts.tile([128, 512], BF16)
        idn_s = consts.tile([128, 128], BF16)
        nc.sync.dma_start(wt_s, wt.ap())
        nc.sync.dma_start(wo_s, wo.ap())
        nc.sync.dma_start(eb_s, eb.ap())
        nc.sync.dma_start(idn_s, idn.ap())

        # sbuf pools
        xt_p = ctx.enter_context(tc.tile_pool(name="xt", bufs=3))
        qk_p = ctx.enter_context(tc.tile_pool(name="qksb", bufs=4))
        v_p = ctx.enter_context(tc.tile_pool(name="vsb", bufs=3))
        e_p = ctx.enter_context(tc.tile_pool(name="esb", bufs=6))
        onm_p = ctx.enter_context(tc.tile_pool(name="onm", bufs=6))
        rc_p = ctx.enter_context(tc.tile_pool(name="rc", bufs=6))
        ot_p = ctx.enter_context(tc.tile_pool(name="ot", bufs=3))
        ob_p = ctx.enter_context(tc.tile_pool(name="ob", bufs=4))

        # psum pools (8 banks of [128, 2KB] total)
        # qkv: [128,512] slots, 2 banks; sp: 4 banks (one per PE row-group --
        # concurrent row-tiled matmuls must write different banks or the
        # device dies); tail: shared 1-bank slots for AV out / O-transpose /
        # out-proj
        qkv_ps = ctx.enter_context(tc.tile_pool(name="qkvps", bufs=2, space="PSUM"))
        sp_ps = ctx.enter_context(tc.tile_pool(name="spps", bufs=1, space="PSUM"))
        tail_ps = ctx.enter_context(tc.tile_pool(name="tailps", bufs=2, space="PSUM"))

        st = {}  # per-group live tiles

        def emit_dma_in(g):
            xt01 = xt_p.tile([128, 1024], BF16, tag="xt01")
            nc.sync.dma_start(xt01, xT.ap()[g])
            st[g] = {"xt": xt01}

        def emit_proj_qk(g, which):  # which: 0 -> q (mt 0,1), 1 -> k (mt 2,3)
            xt01 = st[g]["xt"]
            xtr = [xt01[:, 0:512], xt01[:, 512:1024]]
            dst = qk_p.tile([128, 1024], BF16, tag="qt" if which == 0 else "kt")
            for mt in (2 * which, 2 * which + 1):
                mps = qkv_ps.tile([128, 512], F32, tag="qkvp")
                for kh in range(2):
                    nc.tensor.matmul(
                        mps,
                        wt_s[:, 768 * kh + 128 * mt:768 * kh + 128 * mt + 128],
                        xtr[kh], start=(kh == 0), stop=(kh == 1))
                nc.scalar.activation(
                    dst[:, 512 * (mt % 2):512 * (mt % 2) + 512], mps,
                    mybir.ActivationFunctionType.Copy)
            st[g]["qt" if which == 0 else "kt"] = dst

        def emit_proj_v(g):
            xt01 = st[g]["xt"]
            xtr = [xt01[:, 0:512], xt01[:, 512:1024]]
            va = v_p.tile([128, 1056], BF16, tag="va")  # (t4, h8, 33)
            va_r = va.rearrange("p (t h c) -> p t h c", t=4, h=8, c=33)
            for vh in range(2):  # two tok-tile pairs
                vps = qkv_ps.tile([128, 512], F32, tag="qkvp")
                for t2 in range(2):
                    t = 2 * vh + t2
                    for kh in range(2):
                        nc.tensor.matmul(
                            vps[:, 256 * t2:256 * t2 + 256],
                            xtr[kh][:, 128 * t:128 * t + 128],
                            wt_s[:, 768 * kh + 512:768 * kh + 768],
                            start=(kh == 0), stop=(kh == 1))
                vps_r = vps.rearrange("p (t h c) -> p t h c", t=2, h=8, c=32)
                nc.vector.tensor_copy(va_r[:, 2 * vh:2 * vh + 2, :, 0:32], vps_r)
            nc.gpsimd.memset(va_r[:, :, :, 32], 1.0)
            st[g]["va"] = va

        def emit_scores(g, j):
            qt, kt = st[g]["qt"], st[g]["kt"]
            # scores S^T[(w,kj), (h4-bank: hh,qi)] for window pair j.
            # one psum BANK per PE row-group (h4): concurrent row-tiled
            # matmuls into one bank are fatal on TRN2.
            sp = sp_ps.tile([128, 2048], F32, tag="sp")
            for h in range(HEADS):
                hh, h4 = divmod(h, 4)
                for w in range(2):
                    col = 512 * hh + 128 * j + 64 * w
                    nc.tensor.matmul(
                        sp[64 * w:64 * w + 64,
                           512 * h4 + 64 * hh:512 * h4 + 64 * hh + 64],
                        kt[32 * h4:32 * h4 + 32, col:col + 64],
                        qt[32 * h4:32 * h4 + 32, col:col + 64],
                        start=True, stop=True,
                        tile_position=(32 * h4, 64 * w))

            # E = exp(S) * exp(bias), split in halves (h4 0,1 | h4 2,3) so
            # AV can start on the first half while the second is in flight
            eraw = e_p.tile([128, 512], BF16, tag="eraw")
            et = e_p.tile([128, 512], BF16, tag="et")
            sp_r = sp.rearrange("p (h4 r) -> p h4 r", h4=4)
            er_r = eraw.rearrange("p (hp h4 r) -> p hp h4 r", hp=2, h4=2)
            for half in range(2):
                nc.scalar.activation(
                    er_r[:, half], sp_r[:, 2 * half:2 * half + 2, 0:128],
                    mybir.ActivationFunctionType.Exp)
                nc.vector.tensor_mul(et[:, 256 * half:256 * half + 256],
                                     eraw[:, 256 * half:256 * half + 256],
                                     eb_s[:, 256 * half:256 * half + 256])
            st[g][f"et{j}"] = et

        def emit_av(g, j):
            et = st[g][f"et{j}"]
            va = st[g]["va"]
            va_r = va.rearrange("p (t h c) -> p t h c", t=4, h=8, c=33)
            # AV with ones-augmented V: O[(w,qi), (h, 32d+denom)]
            on = tail_ps.tile([128, 512], F32, tag="tail")
            for h in range(HEADS):
                hh, h4 = divmod(h, 4)
                ecol = 64 * (2 * h4 + hh)
                for w in range(2):
                    nc.tensor.matmul(
                        on[64 * w:64 * w + 64, 33 * h:33 * h + 33],
                        et[64 * w:64 * w + 64, ecol:ecol + 64],
                        va_r[64 * w:64 * w + 64, j, h, :],
                        start=True, stop=True,
                        tile_position=(64 * w, 64 * w))

            on_r = on[:, 0:264].rearrange("p (h c) -> p h c", h=8, c=33)
            rc = rc_p.tile([128, 8], F32, tag="rc")
            nc.vector.reciprocal(rc, on_r[:, :, 32])
            onm = onm_p.tile([128, 256], BF16, tag="onm")
            onm_r = onm.rearrange("p (h c) -> p h c", h=8, c=32)
            in0, in1 = bass.broadcast_tensor_aps(
                on_r[:, :, 0:32], rc.rearrange("p (h o) -> p h o", o=1))
            nc.vector.tensor_tensor(onm_r, in0, in1, op=mybir.AluOpType.mult)
            st[g][f"onm{j}"] = onm

        def emit_tr(g, j):
            onm = st[g][f"onm{j}"]
            if "ot" not in st[g]:
                st[g]["ot"] = ot_p.tile([128, 1024], BF16, tag="ot",
                                        name="ot")
            ot = st[g]["ot"]
            otp = tail_ps.tile([128, 256], BF16, tag="tail")
            for ch_half in range(2):
                nc.tensor.transpose(
                    otp[:, 128 * ch_half:128 * ch_half + 128],
                    onm[:, 128 * ch_half:128 * ch_half + 128],
                    idn_s)
            ot_r = ot.rearrange("p (chh tok) -> p chh tok", chh=2)
            otp_r = otp.rearrange("p (chh tok) -> p chh tok", chh=2)
            nc.vector.tensor_copy(ot_r[:, :, 128 * j:128 * j + 128], otp_r)

        def emit_outproj(g):
            ot = st[g]["ot"]
            ot_r = ot.rearrange("p (chh tok) -> p chh tok", chh=2)
            ob = ob_p.tile([128, 1024], BF16, tag="ob")
            for oh in range(2):
                op = tail_ps.tile([128, 512], F32, tag="tail")
                for kh in range(2):
                    nc.tensor.matmul(
                        op,
                        wo_s[:, 256 * kh + 128 * oh:256 * kh + 128 * oh + 128],
                        ot_r[:, kh, :],
                        start=(kh == 0), stop=(kh == 1))
                if oh == 0:
                    nc.scalar.activation(ob[:, 0:512], op,
                                         mybir.ActivationFunctionType.Copy)
                else:
                    nc.vector.tensor_copy(ob[:, 512:1024], op)
            nc.sync.dma_start(out.ap()[g], ob)
            del st[g]

        # ---- software-pipelined group loop
        emit_dma_in(0)
        emit_proj_qk(0, 0)
        emit_proj_qk(0, 1)
        emit_proj_v(0)
        for g in range(n_groups):
            nxt = g + 1
            if nxt < n_groups:
                emit_dma_in(nxt)
            for j in range(4):
                emit_scores(g, j)
                if j >= 1:
                    emit_av(g, j - 1)
                if j >= 2:
                    emit_tr(g, j - 2)
                if nxt < n_groups:
                    if j == 0:
                        emit_proj_qk(nxt, 0)
                    elif j == 1:
                        emit_proj_qk(nxt, 1)
                    elif j == 2:
                        emit_proj_v(nxt)
            emit_av(g, 3)
            emit_tr(g, 2)
            emit_tr(g, 3)
            emit_outproj(g)

    nc.compile()
    return nc


def _host_prep(x, W_qkv, W_out, b_out, pos_emb):
    b, l, H, W, _ = x.shape
    # xT: per core [32, 128, (kh 2, 512)] channel-major, window-pair tok order
    xr = x.reshape(b * l, 16, WS, 2, 4, 2, WS, 2, 128)
    # dims: [core, wr, r, half, j, w, c, kh, p]
    xt = np.ascontiguousarray(xr.transpose(0, 1, 3, 8, 7, 4, 5, 2, 6))
    # -> [core, wr, half, p, kh, j, w, r, c]
    xt = xt.reshape(b * l, NG_FULL, 128, 1024).astype(BF)

    wq = np.concatenate([W_qkv[:, :256] * SCALE, W_qkv[:, 256:]], axis=1)
    wt = np.ascontiguousarray(
        wq.reshape(2, 128, 768).transpose(1, 0, 2).reshape(128, 1536))

    wo = np.ascontiguousarray(
        W_out.reshape(2, 128, 256).transpose(1, 0, 2).reshape(128, 512)
    ).astype(BF)

    bias = pos_emb.reshape(-1)[_bias_table().reshape(-1)].reshape(64, 64)
    ebt = np.tile(np.exp(bias.T), (2, 8)).astype(BF)

    idn = np.eye(128, dtype=np.float32).astype(BF)
    return xt, wt.astype(BF), wo, ebt, idn


_NC_CACHE = {}


def kernel(x, W_qkv, W_out, b_out, pos_emb):
    x = np.asarray(x, dtype=np.float32)
    W_qkv = np.asarray(W_qkv, dtype=np.float32)
    W_out = np.asarray(W_out, dtype=np.float32)
    b_out = np.asarray(b_out, dtype=np.float32)
    pos_emb = np.asarray(pos_emb, dtype=np.float32)

    b, l, H, W, _ = x.shape
    xt, wt, wo, ebt, idn = _host_prep(x, W_qkv, W_out, b_out, pos_emb)

    if "nc" not in _NC_CACHE:
        _NC_CACHE["nc"] = build()
    nc = _NC_CACHE["nc"]

    in_maps = [
        {"xT": np.ascontiguousarray(xt[i]), "wt": wt, "wo": wo,
         "eb": ebt, "idn": idn}
        for i in range(N_CORES)
    ]
    res = run_bass_kernel_spmd(
        nc, in_maps, list(range(N_CORES)),
        trace=bool(int(os.environ.get("KERNEL_TRACE", "0"))))
    if res.exec_time_ns is not None:
        print(f"HW exec time: {res.exec_time_ns} ns")
    outs = np.stack([res.results[i]["out"] for i in range(N_CORES)])
    return (_unscramble(outs) + b_out).reshape(b, l, H, W, DIM)


def _unscramble(o):
    # [cores, 32(wr,half), 128(outch-in-half p), 1024(oh, j, w, r, c)]
    #   -> [cores, 128, 128, 256]
    n = o.shape[0]
    o = o.astype(np.float32).reshape(n, 16, 2, 128, 2, 4, 2, WS, WS)
    # dims: n, wr, half, p, oh, j, w, r, c
    o = o.transpose(0, 1, 7, 2, 5, 6, 8, 4, 3)  # n wr r half j w c oh p
    return np.ascontiguousarray(o).reshape(n, 128, 128, DIM)


if __name__ == "__main__":
    # quick smoke: run on hardware with random inputs
    rng = np.random.default_rng(0)
    x = rng.standard_normal((2, 4, 128, 128, 256), dtype=np.float32)
    W_qkv = rng.standard_normal((256, 768), dtype=np.float32) * DIM ** -0.5
    W_out = rng.standard_normal((256, 256), dtype=np.float32) * 256 ** -0.5
    b_out = rng.standard_normal(256, dtype=np.float32) * 0.02
    pos_emb = rng.standard_normal((15, 15), dtype=np.float32)
    o = kernel(x=x, W_qkv=W_qkv, W_out=W_out, b_out=b_out, pos_emb=pos_emb)
    print(o.shape, o.dtype)


# revision 23
# speedup vs baseline: 1.4471x; 1.0712x over previous
"""Bass/Trainium2 kernel for BaseWindowAttention (8x8 windows, 8 heads, dim 256).

Data-parallel over 8 NeuronCores: each core processes one (b, l) image of
[128, 128, 256]. Fully fused on-device pipeline: qkv projection -> windowed
attention (64-token windows) -> output projection.

Layout strategy per core:
  - tokens are processed in "groups" of 512 = 4 window-pairs (wp = 2 adjacent
    8x8 windows = 128 tokens, partition order (w, r, c)).
  - x is pre-transposed on host to [32 groups, 128 ch, (2 ch-half, 512 tok)]
    so the contraction dim (channels) lands on SBUF partitions with zero
    on-chip transposes for the projection stage.
  - qT/kT live as [outch(4 heads x 32d) partitions, tok free] -> attention
    score matmuls S^T = kT^T @ qT slice straight out of SBUF via
    tile_position packing (K=32, M=64, N=64).
  - the 64x64 relative-position bias is ADDED into the scores PSUM by one
    extra matmul per (j, h4-bank): stationary = tiled bias^T, moving = tiled
    identity. The exp() then directly produces E = exp(S + B), removing the
    separate bias multiply from the critical path.
  - AV uses E as the stationary operand and V-natural as moving, with a ones
    column appended to V so each window-head matmul also emits the softmax
    denominator. Output lands token-major -> normalize is a native
    per-partition broadcast multiply.
  - O is PE-transposed back to [ch, tok]; the out projection runs
    weight-stationary (W_out^T slices) producing ch-major output written
    straight to DRAM by one DMA per group.
  - the group loop is software-pipelined: group g+1's qkv projections are
    emitted interleaved with group g's attention so the in-order engine
    streams (PE/ACT/DVE) have fill work while the per-j dependency chain
    (scores -> exp -> AV -> normalize -> transpose) crosses engines.
"""

import os
import numpy as np
import ml_dtypes

import concourse.bass as bass
import concourse.bacc as bacc
import concourse.mybir as mybir
import concourse.tile as tile
from concourse.bass_utils import run_bass_kernel_spmd
from contextlib import ExitStack

F32 = mybir.dt.float32
BF16 = mybir.dt.bfloat16

WS = 8
HEADS = 8
HD = 32
DIM = 256
STRIDE = 2 * WS - 1
SCALE = HD ** -0.5
N_CORES = 8
NG_FULL = 32  # 512-token groups per core

BF = ml_dtypes.bfloat16


def _bias_table() -> np.ndarray:
    # bias[qi, kj] from the 15x15 pos table, same as the reference
    coords = np.array([[x, y] for x in range(WS) for y in range(WS)], dtype=np.int32)
    rel = coords[None, :, :] - coords[:, None, :] + (WS - 1)
    idx = rel[:, :, 0] * STRIDE + rel[:, :, 1]
    return np.clip(idx, 0, None).reshape(WS * WS, WS * WS)


def build(n_groups: int = NG_FULL):
    nc = bacc.Bacc("TRN2", target_bir_lowering=False, debug=False,
                   num_devices=N_CORES)

    xT = nc.dram_tensor("xT", [NG_FULL, 128, 1024], BF16, kind="ExternalInput")
    wt = nc.dram_tensor("wt", [128, 1536], BF16, kind="ExternalInput")
    wo = nc.dram_tensor("wo", [128, 512], BF16, kind="ExternalInput")
    eb = nc.dram_tensor("eb", [128, 512], BF16, kind="ExternalInput")
    idn = nc.dram_tensor("idn", [128, 128], BF16, kind="ExternalInput")
    # output in kernel visit order [g, outch-half(p), (oh, tok)]; host
    # inverse-permutes to the natural image layout
    out = nc.dram_tensor("out", [NG_FULL, 128, 1024], BF16,
                         kind="ExternalOutput")

    with tile.TileContext(nc) as tc, ExitStack() as ctx:
        consts = ctx.enter_context(tc.tile_pool(name="consts", bufs=1))
        wt_s = consts.tile([128, 1536], BF16)
        wo_s = consts.tile([128, 512], BF16)
        eb_s = consts.tile([128, 512], BF16)
        idn_s = consts.tile([128, 128], BF16)
        nc.sync.dma_start(wt_s, wt.ap())
        nc.sync.dma_start(wo_s, wo.ap())
        nc.sync.dma_start(eb_s, eb.ap())
        nc.sync.dma_start(idn_s, idn.ap())

        # sbuf pools
        xt_p = ctx.enter_context(tc.tile_pool(name="xt", bufs=3))
        qk_p = ctx.enter_context(tc.tile_pool(name="qksb", bufs=4))
        v_p = ctx.enter_context(tc.tile_pool(name="vsb", bufs=3))
        e_p = ctx.enter_context(tc.tile_pool(name="esb", bufs=6))
        onm_p = ctx.enter_context(tc.tile_pool(name="onm", bufs=6))
        rc_p = ctx.enter_context(tc.tile_pool(name="rc", bufs=6))
        ot_p = ctx.enter_context(tc.tile_pool(name="ot", bufs=3))
        ob_p = ctx.enter_context(tc.tile_pool(name="ob", bufs=4))

        # psum pools (8 banks of [128, 2KB] total)
        # qkv: [128,512] slots, 2 banks; sp: 4 banks (one per PE row-group --
        # concurrent row-tiled matmuls must write different banks or the
        # device dies); tail: shared 1-bank slots for AV out / O-transpose /
        # out-proj
        qkv_ps = ctx.enter_context(tc.tile_pool(name="qkvps", bufs=2, space="PSUM"))
        sp_ps = ctx.enter_context(tc.tile_pool(name="spps", bufs=1, space="PSUM"))
        tail_ps = ctx.enter_context(tc.tile_pool(name="tailps", bufs=2, space="PSUM"))

        st = {}  # per-group live tiles

        def emit_dma_in(g):
            xt01 = xt_p.tile([128, 1024], BF16, tag="xt01")
            nc.sync.dma_start(xt01, xT.ap()[g])
            st[g] = {"xt": xt01}

        def emit_proj_qk(g, which):  # which: 0 -> q (mt 0,1), 1 -> k (mt 2,3)
            xt01 = st[g]["xt"]
            xtr = [xt01[:, 0:512], xt01[:, 512:1024]]
            dst = qk_p.tile([128, 1024], BF16, tag="qt" if which == 0 else "kt")
            for mt in (2 * which, 2 * which + 1):
                mps = qkv_ps.tile([128, 512], F32, tag="qkvp")
                for kh in range(2):
                    nc.tensor.matmul(
                        mps,
                        wt_s[:, 768 * kh + 128 * mt:768 * kh + 128 * mt + 128],
                        xtr[kh], start=(kh == 0), stop=(kh == 1))
                nc.scalar.activation(
                    dst[:, 512 * (mt % 2):512 * (mt % 2) + 512], mps,
                    mybir.ActivationFunctionType.Copy)
            st[g]["qt" if which == 0 else "kt"] = dst

        def emit_proj_v(g):
            xt01 = st[g]["xt"]
            xtr = [xt01[:, 0:512], xt01[:, 512:1024]]
            va = v_p.tile([128, 1056], BF16, tag="va")  # (t4, h8, 33)
            va_r = va.rearrange("p (t h c) -> p t h c", t=4, h=8, c=33)
            for vh in range(2):  # two tok-tile pairs
                vps = qkv_ps.tile([128, 512], F32, tag="qkvp")
                for t2 in range(2):
                    t = 2 * vh + t2
                    for kh in range(2):
                        nc.tensor.matmul(
                            vps[:, 256 * t2:256 * t2 + 256],
                            xtr[kh][:, 128 * t:128 * t + 128],
                            wt_s[:, 768 * kh + 512:768 * kh + 768],
                            start=(kh == 0), stop=(kh == 1))
                vps_r = vps.rearrange("p (t h c) -> p t h c", t=2, h=8, c=32)
                nc.vector.tensor_copy(va_r[:, 2 * vh:2 * vh + 2, :, 0:32], vps_r)
            nc.gpsimd.memset(va_r[:, :, :, 32], 1.0)
            st[g]["va"] = va

        def emit_scores(g, j):
            qt, kt = st[g]["qt"], st[g]["kt"]
            # scores S^T[(w,kj), (h4-bank: hh,qi)] for window pair j.
            # one psum BANK per PE row-group (h4): concurrent row-tiled
            # matmuls into one bank are fatal on TRN2.
            sp = sp_ps.tile([128, 2048], F32, tag="sp")
            for h in range(HEADS):
                hh, h4 = divmod(h, 4)
                for w in range(2):
                    col = 512 * hh + 128 * j + 64 * w
                    nc.tensor.matmul(
                        sp[64 * w:64 * w + 64,
                           512 * h4 + 64 * hh:512 * h4 + 64 * hh + 64],
                        kt[32 * h4:32 * h4 + 32, col:col + 64],
                        qt[32 * h4:32 * h4 + 32, col:col + 64],
                        start=True, stop=True,
                        tile_position=(32 * h4, 64 * w))

            # E = exp(S) * exp(bias), split in halves (h4 0,1 | h4 2,3) so
            # AV can start on the first half while the second is in flight
            eraw = e_p.tile([128, 512], BF16, tag="eraw")
            et = e_p.tile([128, 512], BF16, tag="et")
            sp_r = sp.rearrange("p (h4 r) -> p h4 r", h4=4)
            er_r = eraw.rearrange("p (hp h4 r) -> p hp h4 r", hp=2, h4=2)
            for half in range(2):
                nc.scalar.activation(
                    er_r[:, half], sp_r[:, 2 * half:2 * half + 2, 0:128],
                    mybir.ActivationFunctionType.Exp)
                nc.vector.tensor_mul(et[:, 256 * half:256 * half + 256],
                                     eraw[:, 256 * half:256 * half + 256],
                                     eb_s[:, 256 * half:256 * half + 256])
            st[g][f"et{j}"] = et

        def emit_av(g, j):
            et = st[g][f"et{j}"]
            va = st[g]["va"]
            va_r = va.rearrange("p (t h c) -> p t h c", t=4, h=8, c=33)
            # AV with ones-augmented V: O[(w,qi), (h, 32d+denom)].
            # h4 0,1 first: they only need the first exp/mult half.
            on = tail_ps.tile([128, 512], F32, tag="tail")
            for h in (0, 1, 4, 5, 2, 3, 6, 7):
                hh, h4 = divmod(h, 4)
                ecol = 64 * (2 * h4 + hh)
                for w in range(2):
                    nc.tensor.matmul(
                        on[64 * w:64 * w + 64, 33 * h:33 * h + 33],
                        et[64 * w:64 * w + 64, ecol:ecol + 64],
                        va_r[64 * w:64 * w + 64, j, h, :],
                        start=True, stop=True,
                        tile_position=(64 * w, 64 * w))

            on_r = on[:, 0:264].rearrange("p (h c) -> p h c", h=8, c=33)
            rc = rc_p.tile([128, 8], F32, tag="rc")
            nc.vector.reciprocal(rc, on_r[:, :, 32])
            onm = onm_p.tile([128, 256], BF16, tag="onm")
            onm_r = onm.rearrange("p (h c) -> p h c", h=8, c=32)
            in0, in1 = bass.broadcast_tensor_aps(
                on_r[:, :, 0:32], rc.rearrange("p (h o) -> p h o", o=1))
            nc.vector.tensor_tensor(onm_r, in0, in1, op=mybir.AluOpType.mult)
            st[g][f"onm{j}"] = onm

        def emit_tr(g, j):
            onm = st[g][f"onm{j}"]
            if "ot" not in st[g]:
                st[g]["ot"] = ot_p.tile([128, 1024], BF16, tag="ot",
                                        name="ot")
            ot = st[g]["ot"]
            otp = tail_ps.tile([128, 256], BF16, tag="tail")
            for ch_half in range(2):
                nc.tensor.transpose(
                    otp[:, 128 * ch_half:128 * ch_half + 128],
                    onm[:, 128 * ch_half:128 * ch_half + 128],
                    idn_s)
            ot_r = ot.rearrange("p (chh tok) -> p chh tok", chh=2)
            otp_r = otp.rearrange("p (chh tok) -> p chh tok", chh=2)
            nc.vector.tensor_copy(ot_r[:, :, 128 * j:128 * j + 128], otp_r)

        def emit_outproj(g):
            ot = st[g]["ot"]
            ot_r = ot.rearrange("p (chh tok) -> p chh tok", chh=2)
            ob = ob_p.tile([128, 1024], BF16, tag="ob")
            for oh in range(2):
                op = tail_ps.tile([128, 512], F32, tag="tail")
                for kh in range(2):
                    nc.tensor.matmul(
                        op,
                        wo_s[:, 256 * kh + 128 * oh:256 * kh + 128 * oh + 128],
                        ot_r[:, kh, :],
                        start=(kh == 0), stop=(kh == 1))
                if oh == 0:
                    nc.scalar.activation(ob[:, 0:512], op,
                                         mybir.ActivationFunctionType.Copy)
                else:
                    nc.vector.tensor_copy(ob[:, 512:1024], op)
            nc.sync.dma_start(out.ap()[g], ob)
            del st[g]

        # ---- software-pipelined group loop; group g's tail (last AV /
        # transposes / out-projection) is deferred into group g+1's first
        # slot so next-group scores fill the tail's cross-engine waits
        emit_dma_in(0)
        emit_proj_qk(0, 0)
        emit_proj_qk(0, 1)
        emit_proj_v(0)
        for g in range(n_groups):
            nxt = g + 1
            if nxt < n_groups:
                emit_dma_in(nxt)
            for j in range(4):
                emit_scores(g, j)
                if j == 0 and g > 0:
                    emit_av(g - 1, 3)
                    emit_tr(g - 1, 2)
                    emit_tr(g - 1, 3)
                    emit_outproj(g - 1)
                if j >= 1:
                    emit_av(g, j - 1)
                if j >= 2:
                    emit_tr(g, j - 2)
                if nxt < n_groups:
                    if j == 0:
                        emit_proj_qk(nxt, 0)
                    elif j == 1:
                        emit_proj_qk(nxt, 1)
                    elif j == 2:
                        emit_proj_v(nxt)
        g = n_groups - 1
        emit_av(g, 3)
        emit_tr(g, 2)
        emit_tr(g, 3)
        emit_outproj(g)

    nc.compile()
    return nc


def _host_prep(x, W_qkv, W_out, b_out, pos_emb):
    b, l, H, W, _ = x.shape
    # xT: per core [32, 128, (kh 2, 512)] channel-major, window-pair tok order
    xr = x.reshape(b * l, 16, WS, 2, 4, 2, WS, 2, 128)
    # dims: [core, wr, r, half, j, w, c, kh, p]
    xt = np.ascontiguousarray(xr.transpose(0, 1, 3, 8, 7, 4, 5, 2, 6))
    # -> [core, wr, half, p, kh, j, w, r, c]
    xt = xt.reshape(b * l, NG_FULL, 128, 1024).astype(BF)

    wq = np.concatenate([W_qkv[:, :256] * SCALE, W_qkv[:, 256:]], axis=1)
    wt = np.ascontiguousarray(
        wq.reshape(2, 128, 768).transpose(1, 0, 2).reshape(128, 1536))

    wo = np.ascontiguousarray(
        W_out.reshape(2, 128, 256).transpose(1, 0, 2).reshape(128, 512)
    ).astype(BF)

    bias = pos_emb.reshape(-1)[_bias_table().reshape(-1)].reshape(64, 64)
    ebt = np.tile(np.exp(bias.T), (2, 8)).astype(BF)

    idn = np.eye(128, dtype=np.float32).astype(BF)
    return xt, wt.astype(BF), wo, ebt, idn


_NC_CACHE = {}


def kernel(x, W_qkv, W_out, b_out, pos_emb):
    x = np.asarray(x, dtype=np.float32)
    W_qkv = np.asarray(W_qkv, dtype=np.float32)
    W_out = np.asarray(W_out, dtype=np.float32)
    b_out = np.asarray(b_out, dtype=np.float32)
    pos_emb = np.asarray(pos_emb, dtype=np.float32)

    b, l, H, W, _ = x.shape
    xt, wt, wo, ebt, idn = _host_prep(x, W_qkv, W_out, b_out, pos_emb)

    if "nc" not in _NC_CACHE:
        _NC_CACHE["nc"] = build()
    nc = _NC_CACHE["nc"]

    in_maps = [
        {"xT": np.ascontiguousarray(xt[i]), "wt": wt, "wo": wo,
         "eb": ebt, "idn": idn}
        for i in range(N_CORES)
    ]
    res = run_bass_kernel_spmd(
        nc, in_maps, list(range(N_CORES)),
        trace=bool(int(os.environ.get("KERNEL_TRACE", "0"))))
    if res.exec_time_ns is not None:
        print(f"HW exec time: {res.exec_time_ns} ns")
    outs = np.stack([res.results[i]["out"] for i in range(N_CORES)])
    return (_unscramble(outs) + b_out).reshape(b, l, H, W, DIM)


def _unscramble(o):
    # [cores, 32(wr,half), 128(outch-in-half p), 1024(oh, j, w, r, c)]
    #   -> [cores, 128, 128, 256]
    n = o.shape[0]
    o = o.astype(np.float32).reshape(n, 16, 2, 128, 2, 4, 2, WS, WS)
    # dims: n, wr, half, p, oh, j, w, r, c
    o = o.transpose(0, 1, 7, 2, 5, 6, 8, 4, 3)  # n wr r half j w c oh p
    return np.ascontiguousarray(o).reshape(n, 128, 128, DIM)


if __name__ == "__main__":
    # quick smoke: run on hardware with random inputs
    rng = np.random.default_rng(0)
    x = rng.standard_normal((2, 4, 128, 128, 256), dtype=np.float32)
    W_qkv = rng.standard_normal((256, 768), dtype=np.float32) * DIM ** -0.5
    W_out = rng.standard_normal((256, 256), dtype=np.float32) * 256 ** -0.5
    b_out = rng.standard_normal(256, dtype=np.float32) * 0.02
    pos_emb = rng.standard_normal((15, 15), dtype=np.float32)
    o = kernel(x=x, W_qkv=W_qkv, W_out=W_out, b_out=b_out, pos_emb=pos_emb)
    print(o.shape, o.dtype)
